# revision 8
# baseline (speedup 1.0000x reference)
"""Trainium2 Bass kernel for the MAB dense-transformer block.

Sharding: 8 cores = 2 batches x 4 Sq-slices (512 each). Each core:
  - projects k = wk @ K_b, vT = (wv @ K_b)^T for its whole batch (replicated
    across the 4 cores sharing the batch; no collectives needed),
  - q = (wq * dk^-0.5) @ Q_b[:, slice],
  - per head: logits^T = k_h^T q_h  ([Sk, Sq_loc] tiles), sigmoid on ACT,
    o^T accumulated via PE with a ones-column in vT giving row-sums for the
    renormalization for free, division via PE broadcast of 1/s,
  - proj (wp) + Q residual, FFN (relu(w1 x + b1) -> w2) + residual.
All matmuls bf16 with fp32 PSUM accumulation. Weights pre-tiled on host.
"""

import numpy as np
import ml_dtypes

BF = ml_dtypes.bfloat16

B, DIM, H, DK, SQ, SK = 2, 1024, 16, 64, 2048, 2048
D = H * DK
NCORES = 8
QSL = SQ // 4          # 512 columns of Sq per core
NG = 4                 # head groups per core (4 heads each)
GH = H // NG

_nc_cache = {}


def _build_nc(mask_ones, bq_nz, bk_nz, bp_nz, b2_nz, dbg=False):
    from concourse import bacc, mybir
    import concourse.tile as tile

    bf16 = mybir.dt.bfloat16
    f32 = mybir.dt.float32
    AF = mybir.ActivationFunctionType

    nc = bacc.Bacc("TRN2")

    d_Kb = nc.declare_dram_parameter("Kb", [DIM, SK], bf16, isOutput=False)
    d_Qb = nc.declare_dram_parameter("Qb", [DIM, QSL], bf16, isOutput=False)
    d_Qres = nc.declare_dram_parameter("Qres", [DIM, QSL], f32, isOutput=False)
    d_wq = nc.declare_dram_parameter("wq", [8, 128, 8, 128], bf16, isOutput=False)
    d_wk = nc.declare_dram_parameter("wk", [8, 128, 8, 128], bf16, isOutput=False)
    d_wv = nc.declare_dram_parameter("wv", [8, 128, D], bf16, isOutput=False)
    d_wp = nc.declare_dram_parameter("wp", [8, 128, 8, 128], bf16, isOutput=False)
    d_w1 = nc.declare_dram_parameter("w1", [16, 128, 8, 128], bf16, isOutput=False)
    d_w2 = nc.declare_dram_parameter("w2", [8, 128, 16, 128], bf16, isOutput=False)
    d_b1 = nc.declare_dram_parameter("b1t", [128, 16], f32, isOutput=False)
    d_bq = d_bk = d_bp = d_b2 = d_madd = None
    if bq_nz:
        d_bq = nc.declare_dram_parameter("bqt", [128, 8], f32, isOutput=False)
    if bk_nz:
        d_bk = nc.declare_dram_parameter("bkt", [128, 8], f32, isOutput=False)
    if bp_nz:
        d_bp = nc.declare_dram_parameter("bpt", [128, 8], f32, isOutput=False)
    if b2_nz:
        d_b2 = nc.declare_dram_parameter("b2t", [128, 8], f32, isOutput=False)
    if not mask_ones:
        d_madd = nc.declare_dram_parameter("maddt", [128, 16], f32, isOutput=False)
    d_out = nc.declare_dram_parameter("out", [DIM, QSL], f32, isOutput=True)
    if dbg:
        d_dq = nc.declare_dram_parameter("dbg_q", [128, 8, QSL], bf16, isOutput=True)
        d_dk = nc.declare_dram_parameter("dbg_kg", [128, 4, SK], bf16, isOutput=True)
        d_dv = nc.declare_dram_parameter("dbg_vt", [128, 16, GH, 65], bf16, isOutput=True)
        d_do = nc.declare_dram_parameter("dbg_o", [128, 8, QSL], bf16, isOutput=True)

    with tile.TileContext(nc) as tc:
        with (
            tc.tile_pool(name="pin", bufs=1) as pin,
            tc.tile_pool(name="pw", bufs=3) as pw,
            tc.tile_pool(name="pkv", bufs=2) as pkv,
            tc.tile_pool(name="pq", bufs=1) as pq,
            tc.tile_pool(name="pwt", bufs=4) as pwt,
            tc.tile_pool(name="po", bufs=1) as po,
            tc.tile_pool(name="ph", bufs=1) as ph,
            tc.tile_pool(name="psmall", bufs=2) as psmall,
            tc.tile_pool(name="pconst", bufs=1) as pconst,
            tc.tile_pool(name="pout", bufs=2) as pout,
            tc.tile_pool(name="ppsA", bufs=4, space="PSUM") as ppsA,
            tc.tile_pool(name="ppsO", bufs=4, space="PSUM") as ppsO,
        ):
            # ---- input loads (qb first so q-proj starts early; kb/wv chunked) ----
            qb = pin.tile([128, 8, QSL], bf16, tag="qb")
            qbr = d_Qb[:].rearrange("(c p) s -> p c s", p=128)
            for c in range(8):
                nc.sync.dma_start(out=qb[:, c, :], in_=qbr[:, c, :])
            kb = pin.tile([128, 8, SK], bf16, tag="kb")
            kbr = d_Kb[:].rearrange("(c p) s -> p c s", p=128)
            for c in range(8):
                nc.sync.dma_start(out=kb[:, c, :], in_=kbr[:, c, :])
            wv_sb = pin.tile([128, 8, D], bf16, tag="wv")
            for c in range(8):
                nc.sync.dma_start(out=wv_sb[:, c, :], in_=d_wv[c])
            b1_sb = pconst.tile([128, 16], f32, tag="b1")
            nc.sync.dma_start(out=b1_sb, in_=d_b1[:])
            ones_row = pconst.tile([1, 64], f32, tag="ones")
            nc.vector.memset(ones_row, 1.0)
            bq_sb = bk_sb = bp_sb = b2_sb = madd_sb = None
            if bq_nz:
                bq_sb = pconst.tile([128, 8], f32, tag="bq")
                nc.sync.dma_start(out=bq_sb, in_=d_bq[:])
            if bk_nz:
                bk_sb = pconst.tile([128, 8], f32, tag="bk")
                nc.sync.dma_start(out=bk_sb, in_=d_bk[:])
            if bp_nz:
                bp_sb = pconst.tile([128, 8], f32, tag="bp")
                nc.sync.dma_start(out=bp_sb, in_=d_bp[:])
            if b2_nz:
                b2_sb = pconst.tile([128, 8], f32, tag="b2")
                nc.sync.dma_start(out=b2_sb, in_=d_b2[:])
            if not mask_ones:
                madd_sb = pconst.tile([128, 16], f32, tag="madd")
                nc.sync.dma_start(out=madd_sb, in_=d_madd[:])

            # ---- q projection: q_sb[p, m, :] (bf16, pre-scaled weights) ----
            q_sb = pq.tile([128, 8, QSL], bf16, tag="q")
            for m in range(8):
                wt = pw.tile([128, 8, 128], bf16, tag="w")
                nc.sync.dma_start(out=wt, in_=d_wq[m])
                ps = ppsA.tile([128, QSL], f32, tag="lg")
                for c in range(8):
                    nc.tensor.matmul(
                        ps, wt[:, c, :], qb[:, c, :],
                        start=(c == 0), stop=(c == 7),
                    )
                if bq_nz:
                    nc.scalar.activation(
                        q_sb[:, m, :], ps, AF.Identity, bias=bq_sb[:, m : m + 1]
                    )
                else:
                    nc.vector.tensor_copy(q_sb[:, m, :], ps)

            if dbg:
                nc.sync.dma_start(out=d_dq[:], in_=q_sb)
            o_sb = po.tile([128, 8, QSL], bf16, tag="o")
            o_ff = po.tile([128, 8, QSL], bf16, tag="off")
            o_res = po.tile([128, 8, QSL], f32, tag="ores")

            def emit_kvproj(g):
                """k rows + transposed-v for heads 4g..4g+3; returns (kg, vt, units)."""
                kg = pkv.tile([128, 2, SK], bf16, tag="kg")
                vt = pkv.tile([128, 16, GH, 65], bf16, tag="vg")
                units = []
                def k_unit(lm):
                    m = 2 * g + lm
                    wt = pw.tile([128, 8, 128], bf16, tag="w")
                    nc.sync.dma_start(out=wt, in_=d_wk[m])
                    for n in range(4):
                        ps = ppsA.tile([128, 512], f32, tag="lg")
                        for c in range(8):
                            nc.tensor.matmul(
                                ps, wt[:, c, :], kb[:, c, 512 * n : 512 * n + 512],
                                start=(c == 0), stop=(c == 7),
                            )
                        if bk_nz:
                            nc.scalar.activation(
                                kg[:, lm, 512 * n : 512 * n + 512], ps, AF.Identity,
                                bias=bk_sb[:, m : m + 1],
                            )
                        else:
                            nc.vector.tensor_copy(
                                kg[:, lm, 512 * n : 512 * n + 512], ps
                            )
                def v_unit(t0):
                    for t in range(t0, t0 + 4):
                        nc.vector.memset(vt[:, t, :, 64:65], 1.0)
                        ps = ppsA.tile([128, 512], f32, tag="lg")
                        for c in range(8):
                            nc.tensor.matmul(
                                ps[:, 0:256],
                                kb[:, c, 128 * t : 128 * t + 128],
                                wv_sb[:, c, 256 * g : 256 * g + 256],
                                start=(c == 0), stop=(c == 7),
                            )
                        nc.vector.tensor_copy(
                            vt[:, t, :, 0:64],
                            ps[:, 0:256].rearrange("p (h d) -> p h d", h=GH),
                        )
                units.append(lambda: k_unit(0))
                units.append(lambda: k_unit(1))
                for t0 in (0, 4, 8, 12):
                    units.append(lambda t0=t0: v_unit(t0))
                return kg, vt, units

            def norm_tail(ps_o, r0, oc):
                sc = psmall.tile([1, QSL], f32, tag="sc")
                nc.vector.tensor_copy(sc, ps_o[64:65, :])
                rc = psmall.tile([1, QSL], f32, tag="rc")
                nc.vector.reciprocal_approx_fast(out=rc, in_=sc)
                rb = psmall.tile([64, QSL], f32, tag="rb")
                nc.gpsimd.partition_broadcast(rb, rc)
                nc.vector.tensor_mul(o_sb[r0 : r0 + 64, oc, :], ps_o[0:64, :], rb)

            def emit_head_pair(kg, vt, g, j):
                """heads 4g+2j (rows 0:64) and 4g+2j+1 (rows 64:128) interleaved
                at chunk granularity so both PE row-groups stream concurrently."""
                lm = j
                oc = 2 * g + j
                ps_oE = ppsO.tile([65, QSL], f32, tag="oacc")
                ps_oO = ppsO.tile([65, QSL], f32, tag="oacc")
                for t in range(16):
                    ls = []
                    for r0, tagix in ((0, 0), (64, 1)):
                        ps_l = ppsA.tile([128, 512], f32, tag="lg")
                        nc.tensor.matmul(
                            ps_l,
                            kg[r0 : r0 + 64, lm, 128 * t : 128 * t + 128],
                            q_sb[r0 : r0 + 64, oc, :],
                            start=True, stop=True,
                        )
                        ls.append(ps_l)
                    for (r0, ps_l, ps_o, hl) in (
                        (0, ls[0], ps_oE, 2 * j), (64, ls[1], ps_oO, 2 * j + 1)
                    ):
                        wt_t = pwt.tile([128, 512], bf16, tag="wt")
                        if mask_ones:
                            nc.scalar.activation(wt_t, ps_l, AF.Sigmoid)
                        else:
                            nc.scalar.activation(
                                wt_t, ps_l, AF.Sigmoid, bias=madd_sb[:, t : t + 1]
                            )
                        nc.tensor.matmul(
                            ps_o, vt[:, t, hl, :], wt_t,
                            start=(t == 0), stop=(t == 15),
                        )
                norm_tail(ps_oE, 0, oc)
                norm_tail(ps_oO, 64, oc)

            kg_cur, vt_cur, units0 = emit_kvproj(0)
            for u in units0:
                u()
            for g in range(NG):
                nxt = emit_kvproj(g + 1) if g + 1 < NG else None
                for j in range(2):
                    emit_head_pair(kg_cur, vt_cur, g, j)
                    if nxt is not None:
                        for u in nxt[2][3 * j : 3 * j + 3]:
                            u()
                if nxt is not None:
                    kg_cur, vt_cur = nxt[0], nxt[1]

            # ---- proj + Q residual ----
            for m in range(8):
                wt = pw.tile([128, 8, 128], bf16, tag="w")
                nc.sync.dma_start(out=wt, in_=d_wp[m])
                ps = ppsA.tile([128, QSL], f32, tag="lg")
                for c in range(8):
                    nc.tensor.matmul(
                        ps, wt[:, c, :], o_sb[:, c, :],
                        start=(c == 0), stop=(c == 7),
                    )
                if bp_nz:
                    nc.scalar.activation(ps, ps, AF.Identity, bias=bp_sb[:, m : m + 1])
                qr = psmall.tile([128, QSL], f32, tag="qr")
                nc.sync.dma_start(out=qr, in_=d_Qres[128 * m : 128 * m + 128, :])
                nc.vector.tensor_add(o_res[:, m, :], ps, qr)
                nc.vector.tensor_copy(o_ff[:, m, :], o_res[:, m, :])

            # ---- FFN ----
            h_sb = ph.tile([128, 16, QSL], bf16, tag="h")
            for m in range(16):
                wt = pw.tile([128, 8, 128], bf16, tag="w")
                nc.sync.dma_start(out=wt, in_=d_w1[m])
                ps = ppsA.tile([128, QSL], f32, tag="lg")
                for c in range(8):
                    nc.tensor.matmul(
                        ps, wt[:, c, :], o_ff[:, c, :],
                        start=(c == 0), stop=(c == 7),
                    )
                nc.scalar.activation(
                    h_sb[:, m, :], ps, AF.Relu, bias=b1_sb[:, m : m + 1]
                )
            for m in range(8):
                wt = pw.tile([128, 16, 128], bf16, tag="w")
                nc.sync.dma_start(out=wt, in_=d_w2[m])
                ps = ppsA.tile([128, QSL], f32, tag="lg")
                for c in range(16):
                    nc.tensor.matmul(
                        ps, wt[:, c, :], h_sb[:, c, :],
                        start=(c == 0), stop=(c == 15),
                    )
                if b2_nz:
                    nc.scalar.activation(ps, ps, AF.Identity, bias=b2_sb[:, m : m + 1])
                ot = pout.tile([128, QSL], f32, tag="out")
                nc.vector.tensor_add(ot, ps, o_res[:, m, :])
                nc.sync.dma_start(out=d_out[128 * m : 128 * m + 128, :], in_=ot)

    nc.finalize()
    return nc


def _tile_lhsT(wT, mt, ct):
    # wT [K, M] -> [M/128, 128, K/128, 128] tiles: [m, p, c, j] = wT[128c+p, 128m+j]
    K, M = wT.shape
    a = wT.reshape(K // 128, 128, M // 128, 128)
    return np.ascontiguousarray(a.transpose(2, 1, 0, 3))


def kernel(**inputs):
    np32 = lambda x: np.asarray(x, dtype=np.float32)
    Q = np32(inputs["Q"]); K = np32(inputs["K"]); mask = np32(inputs["mask"])
    wq = np32(inputs["wq"]); bq = np32(inputs["bq"])
    wk = np32(inputs["wk"]); bk = np32(inputs["bk"])
    wv = np32(inputs["wv"]); bv = np32(inputs["bv"])
    wp = np32(inputs["wp"]); bp = np32(inputs["bp"])
    w1 = np32(inputs["w1"]); b1 = np32(inputs["b1"])
    w2 = np32(inputs["w2"]); b2 = np32(inputs["b2"])

    scale = DK ** -0.5
    wq_eff = wq * scale
    bq_eff = bq * scale
    bp_eff = bp + wp @ bv          # fold v bias through the projection

    mask_ones = bool(np.all(mask == 1.0))
    bq_nz = bool(np.any(bq_eff)); bk_nz = bool(np.any(bk))
    bp_nz = bool(np.any(bp_eff)); b2_nz = bool(np.any(b2))

    key = (mask_ones, bq_nz, bk_nz, bp_nz, b2_nz)
    if key not in _nc_cache:
        _nc_cache[key] = _build_nc(*key)
    nc = _nc_cache[key]

    wq_t = _tile_lhsT(wq_eff.T, 8, 8).astype(BF)
    wk_t = _tile_lhsT(wk.T, 8, 8).astype(BF)
    wv_t = np.ascontiguousarray(wv.T.reshape(8, 128, D)).astype(BF)
    wp_t = _tile_lhsT(wp.T, 8, 8).astype(BF)
    w1_t = _tile_lhsT(w1.T, 16, 8).astype(BF)
    w2_t = _tile_lhsT(w2.T, 8, 16).astype(BF)
    b1_t = np.ascontiguousarray(b1.reshape(16, 128).T)

    Kb_bf = [np.ascontiguousarray(K[b]).astype(BF) for b in range(B)]
    madd_t = [
        np.ascontiguousarray((-(1.0 - mask[b, 0]) * 10000.0).reshape(16, 128).T)
        for b in range(B)
    ]

    in_maps = []
    for c in range(NCORES):
        b, s = c // 4, c % 4
        sl = slice(QSL * s, QSL * s + QSL)
        m = {
            "Kb": Kb_bf[b],
            "Qb": np.ascontiguousarray(Q[b][:, sl]).astype(BF),
            "Qres": np.ascontiguousarray(Q[b][:, sl]),
            "wq": wq_t, "wk": wk_t, "wv": wv_t, "wp": wp_t,
            "w1": w1_t, "w2": w2_t, "b1t": b1_t,
        }
        if bq_nz:
            m["bqt"] = np.ascontiguousarray(bq_eff.reshape(8, 128).T)
        if bk_nz:
            m["bkt"] = np.ascontiguousarray(bk.reshape(8, 128).T)
        if bp_nz:
            m["bpt"] = np.ascontiguousarray(bp_eff.reshape(8, 128).T)
        if b2_nz:
            m["b2t"] = np.ascontiguousarray(b2.reshape(8, 128).T)
        if not mask_ones:
            m["maddt"] = madd_t[b]
        in_maps.append(m)

    from concourse.bass_utils import run_bass_kernel_spmd

    res = run_bass_kernel_spmd(nc, in_maps, list(range(NCORES)))

    out = np.empty((B, DIM, SQ), np.float32)
    for c in range(NCORES):
        b, s = c // 4, c % 4
        out[b][:, QSL * s : QSL * s + QSL] = res.results[c]["out"]
    return out
